# revision 1
# baseline (speedup 1.0000x reference)
"""Trainium2 Bass kernel for nn_DENIS_JBF (Koopman Jordan-block forecast model).

Strategy (pure data parallel, 8 cores, Bc = 512 batch per core):
  - BatchNorm batch statistics need the FULL batch: each core computes
    per-shard Gram sums on PE ([X|1]^T [X|1] accumulate), a tiny (2x18x18)
    AllReduce combines them, then BN is folded into the L1 weights on-device.
  - Everything on-chip runs feature-major ("d-major": features on partitions,
    (t, b) flattened on the free dim) so every matmul streams at 1 col/cycle
    with float32r (fp22) inputs.
  - Biases ride the matmuls (ones-row / K=1 accumulate tricks), so each
    leaky-relu eviction is ACT copy + one DVE scalar_tensor_tensor max.
  - The sequential Koopman scan is replaced by its exact closed form
    (complex power of the per-sample rotation), all 64 steps in parallel.
  - x_pred = yl @ Cw^T via PE after transposing yl tiles back to d-major.
  - Host only marshals layouts (shard/transpose/concat); all FLOPs on device.
"""

import os
import sys

import numpy as np

for _p in ("/opt/trn_rl_repo", "/root/.axon_site/_ro/trn_rl_repo"):
    if os.path.isdir(_p) and _p not in sys.path:
        sys.path.insert(0, _p)

import concourse.bass as bass
from concourse import bacc
import concourse.mybir as mybir
import concourse.tile as tile
from concourse import bass_utils

F32 = mybir.dt.float32
F32R = mybir.dt.float32r
AF = mybir.ActivationFunctionType
OP = mybir.AluOpType
AX = mybir.AxisListType

NCORES = 8
B, T, DIM, LDIM, NAUX = 4096, 64, 16, 64, 32
H, AH = 256, 128
DT = 0.01
EPS = 1e-5
BC = B // NCORES            # 512
COLS = BC * T               # 32768 cols, order (t, b): col = t*BC + b
NBLK = COLS // 512          # 64 encoder col-blocks (one per t since BC=512)
RCH = BC // 128             # 4 rotation b-chunks
PI = float(np.pi)
D1 = DIM + 1                # 17: input dim + ones row (bias-in-matmul)


def build():
    nc = bacc.Bacc(None)

    # ---------------- DRAM I/O ----------------
    xsT_d = nc.dram_tensor("xsT", [D1, T, BC], F32R, kind="ExternalInput")
    xsN1_d = nc.dram_tensor("xsN1", [COLS, DIM + 2], F32R, kind="ExternalInput")
    w1e_d = nc.dram_tensor("w1e", [DIM, H], F32, kind="ExternalInput")
    b1r_d = nc.dram_tensor("b1r", [1, H], F32, kind="ExternalInput")
    w2_d = nc.dram_tensor("w2", [128, 4, 128], F32R, kind="ExternalInput")
    b2r_d = nc.dram_tensor("b2r", [1, H], F32R, kind="ExternalInput")
    w3_d = nc.dram_tensor("w3", [128, 2, LDIM], F32R, kind="ExternalInput")
    w1a_d = nc.dram_tensor("w1a", [DIM, NAUX * AH], F32R, kind="ExternalInput")
    b1ar_d = nc.dram_tensor("b1ar", [1, NAUX * AH], F32R, kind="ExternalInput")
    w2a_d = nc.dram_tensor("w2a", [AH, NAUX, AH], F32R, kind="ExternalInput")
    b2ar_d = nc.dram_tensor("b2ar", [1, NAUX * AH], F32R, kind="ExternalInput")
    w3a_d = nc.dram_tensor("w3a", [AH, NAUX, LDIM], F32R, kind="ExternalInput")
    cw2_d = nc.dram_tensor("cw2", [128, 32], F32R, kind="ExternalInput")
    tv_d = nc.dram_tensor("tv", [128, T], F32, kind="ExternalInput")
    id_d = nc.dram_tensor("idm", [128, 128], F32R, kind="ExternalInput")
    onesr_d = nc.dram_tensor("onesr", [1, 512], F32R, kind="ExternalInput")

    yencT_o = nc.dram_tensor("yencT", [LDIM, T, BC], F32, kind="ExternalOutput")
    yl_o = nc.dram_tensor("yl", [BC, T * LDIM], F32R, kind="ExternalOutput")
    xpT_o = nc.dram_tensor("xpT", [32, 32, BC], F32, kind="ExternalOutput")

    stat_in = nc.dram_tensor("stat_in", [2, 18, 18], F32)
    stat_out = nc.dram_tensor("stat_out", [2, 18, 18], F32, addr_space="Shared")

    with tile.TileContext(nc) as tc:
        with tc.tile_pool(name="consts", bufs=1) as cp, \
             tc.tile_pool(name="psum", bufs=1, space="PSUM") as pp, \
             tc.tile_pool(name="stream", bufs=2) as sp, \
             tc.tile_pool(name="chunks", bufs=2) as chp, \
             tc.tile_pool(name="rot", bufs=1) as rp, \
             tc.tile_pool(name="smalls", bufs=1) as smp:

            # ---------------- constants / weights to SBUF ----------------
            identR_t = cp.tile([128, 128], F32R)
            nc.sync.dma_start(out=identR_t, in_=id_d[:, :])
            identR = identR_t[:]
            hpib = cp.tile([128, 1], F32)
            nc.vector.memset(hpib, PI / 2.0)
            epsb = cp.tile([128, 1], F32)
            nc.vector.memset(epsb, EPS)
            ones_row = cp.tile([1, 512], F32R)
            nc.sync.dma_start(out=ones_row, in_=onesr_d[:, :])

            w1e_sb = cp.tile([DIM, H], F32)
            nc.sync.dma_start(out=w1e_sb, in_=w1e_d[:, :])
            b1r_sb = cp.tile([1, H], F32)
            nc.sync.dma_start(out=b1r_sb, in_=b1r_d[:, :])
            w2_sb = cp.tile([128, 4, 128], F32R)
            nc.sync.dma_start(out=w2_sb, in_=w2_d[:, :, :])
            b2r_sb = cp.tile([1, H], F32R)
            nc.sync.dma_start(out=b2r_sb, in_=b2r_d[:, :])
            w3_sb = cp.tile([128, 2, LDIM], F32R)
            nc.sync.dma_start(out=w3_sb, in_=w3_d[:, :, :])

            w2a_sb = cp.tile([AH, NAUX, AH], F32R)
            nc.sync.dma_start(out=w2a_sb, in_=w2a_d[:, :, :])
            b2ar_sb = cp.tile([1, NAUX * AH], F32R)
            nc.sync.dma_start(out=b2ar_sb, in_=b2ar_d[:, :])
            w3a_sb = cp.tile([AH, NAUX, LDIM], F32R)
            nc.sync.dma_start(out=w3a_sb, in_=w3a_d[:, :, :])
            cw2_sb = cp.tile([128, 32], F32R)
            nc.sync.dma_start(out=cw2_sb, in_=cw2_d[:, :])
            tv_sb = cp.tile([128, T], F32)
            nc.sync.dma_start(out=tv_sb, in_=tv_d[:, :])

            # ---------------- phase A: batch-stat Gram sums ----------------
            xsg = xsN1_d[:, :].rearrange("(g p) c -> g p c", p=128)
            pg = pp.tile([18, 18], F32, tag="p1")
            NG = 16
            for i in range(COLS // (128 * NG)):          # 16 batched loads
                ch = chp.tile([128, NG, DIM + 2], F32R, tag="statch", bufs=2)
                nc.gpsimd.dma_start(
                    out=ch,
                    in_=xsg[i * NG:(i + 1) * NG, :, :].transpose([1, 0, 2]))
                for g in range(NG):
                    idx = i * NG + g
                    nc.tensor.matmul(pg[:, :], ch[:, g, :], ch[:, g, :],
                                     start=(idx == 0), stop=(idx == 255))
            x0rows = xsN1_d[:, :].rearrange("(b t) c -> b t c", t=T)[:, 0, :]
            x0g = x0rows.rearrange("(g p) c -> p g c", p=128)
            pg0 = pp.tile([18, 18], F32, tag="p2")
            ch0 = chp.tile([128, RCH, DIM + 2], F32R, tag="statch0", bufs=1)
            nc.gpsimd.dma_start(out=ch0, in_=x0g)
            for g in range(RCH):
                nc.tensor.matmul(pg0[:, :], ch0[:, g, :], ch0[:, g, :],
                                 start=(g == 0), stop=(g == RCH - 1))
            gA = smp.tile([18, 18], F32)
            nc.vector.tensor_copy(gA, pg[:, :])
            gB = smp.tile([18, 18], F32)
            nc.vector.tensor_copy(gB, pg0[:, :])
            nc.sync.dma_start(out=stat_in[0, :, :], in_=gA[:])
            nc.sync.dma_start(out=stat_in[1, :, :], in_=gB[:])
            nc.gpsimd.collective_compute(
                "AllReduce", OP.add, replica_groups=[list(range(NCORES))],
                ins=[stat_in[:, :, :]], outs=[stat_out[:, :, :]])
            stats = smp.tile([18, 2, 18], F32)
            nc.sync.dma_start(out=stats,
                              in_=stat_out[:, :, :].transpose([1, 0, 2]))

            # ---------------- phase A2: fold BN into L1 weights --------------
            def fold(set_idx, n):
                g16 = stats[0:16, set_idx, 0:16]
                scol = stats[0:16, set_idx, 16:17]
                m = smp.tile([DIM, 1], F32, tag=f"m{set_idx}")
                nc.vector.tensor_scalar(m, scol, 1.0 / n, None, OP.mult)
                gi = smp.tile([DIM, DIM], F32, tag=f"gi{set_idx}")
                nc.vector.tensor_mul(gi, g16,
                                     identR_t[0:16, 0:16].bitcast(F32))
                qd = smp.tile([DIM, 1], F32, tag=f"qd{set_idx}")
                nc.vector.reduce_sum(qd, gi, axis=AX.X)
                m2 = smp.tile([DIM, 1], F32, tag=f"m2{set_idx}")
                nc.vector.tensor_mul(m2, m, m)
                v = smp.tile([DIM, 1], F32, tag=f"v{set_idx}")
                nc.vector.scalar_tensor_tensor(v, qd, 1.0 / n, m2,
                                               OP.mult, OP.subtract)
                sd = smp.tile([DIM, 1], F32, tag=f"sd{set_idx}")
                nc.scalar.activation(sd, v, AF.Sqrt, bias=epsb[0:DIM, :])
                rs = smp.tile([DIM, 1], F32, tag=f"rs{set_idx}")
                nc.vector.reciprocal(rs, sd)
                return m, rs

            m_all, rs_all = fold(0, float(B * T))
            m_0, rs_0 = fold(1, float(B))

            # W1ES: [17, 256] f32r; rows 0-15 scaled weights, row 16 = bias
            w1es = cp.tile([D1, H], F32R)
            nc.vector.tensor_mul(w1es[0:DIM, :], w1e_sb,
                                 rs_all[:].broadcast_to([DIM, H]))
            negm = smp.tile([DIM, 2], F32R)
            nc.vector.tensor_scalar(negm, m_all[:].broadcast_to([DIM, 2]),
                                    -1.0, None, OP.mult)
            pbc = pp.tile([2, H], F32, tag="pa2")
            nc.tensor.matmul(pbc[:, :], negm[:], w1es[0:DIM, :],
                             start=True, stop=True)
            brow = smp.tile([1, H], F32R)
            nc.vector.tensor_add(brow, pbc[0:1, :], b1r_sb[:])
            nc.sync.dma_start(out=w1es[DIM:D1, :], in_=brow[:])

            # W1AS: [17, 4096]; rows 0-15 scaled, row 16 = host-folded bias
            w1as = cp.tile([D1, NAUX * AH], F32R)
            nc.sync.dma_start(out=w1as[0:DIM, :], in_=w1a_d[:, :])
            nc.vector.tensor_mul(w1as[0:DIM, :], w1as[0:DIM, :].bitcast(F32),
                                 rs_0[:].broadcast_to([DIM, NAUX * AH]))
            nc.sync.dma_start(out=w1as[DIM:D1, :], in_=b1ar_d[:, :])

            # centered x0 + ones row
            x0T = sp.tile([DIM, BC], F32, tag="x0T", bufs=1)
            nc.sync.dma_start(out=x0T, in_=xsT_d[0:DIM, 0, :].bitcast(F32))
            x0c = cp.tile([D1, BC], F32R)
            nc.vector.tensor_sub(x0c[0:DIM, :], x0T,
                                 m_0[:].broadcast_to([DIM, BC]))
            nc.sync.dma_start(out=x0c[DIM:D1, :], in_=ones_row[:])

            # ---------------- phase B: aux nets -> mu*DT / om*DT -------------
            pmw = pp.tile([LDIM, BC], F32, tag="pmw")
            for kp in range(NAUX // 2):                 # net pairs
                pa1 = pp.tile([128, 2, BC], F32, tag="p1")
                ea = sp.tile([128, 2, BC], F32, tag="ea", bufs=1)
                h1a = sp.tile([128, 2, BC], F32R, tag="h1a", bufs=1)
                for s in range(2):
                    k = kp * 2 + s
                    nc.tensor.matmul(pa1[:, s, :],
                                     w1as[:, k * AH:(k + 1) * AH],
                                     x0c[:], start=True, stop=True)
                nc.scalar.copy(ea[:], pa1[:, :, :])
                nc.vector.scalar_tensor_tensor(h1a[:], ea[:], 0.01, ea[:],
                                               OP.mult, OP.max)
                pa2 = pp.tile([128, 2, BC], F32, tag="p2")
                eb = sp.tile([128, 2, BC], F32, tag="eb", bufs=1)
                h2a = sp.tile([128, 2, BC], F32R, tag="h2a", bufs=1)
                for s in range(2):
                    k = kp * 2 + s
                    nc.tensor.matmul(pa2[:, s, :], w2a_sb[:, k, :],
                                     h1a[:, s, :], start=True, stop=False)
                    nc.tensor.matmul(pa2[:, s, :],
                                     b2ar_sb[:, k * AH:(k + 1) * AH],
                                     ones_row[:], start=False, stop=True)
                nc.scalar.copy(eb[:], pa2[:, :, :])
                nc.vector.scalar_tensor_tensor(h2a[:], eb[:], 0.01, eb[:],
                                               OP.mult, OP.max)
                for s in range(2):
                    k = kp * 2 + s
                    nc.tensor.matmul(pmw[:, :], w3a_sb[:, k, :],
                                     h2a[:, s, :],
                                     start=(k == 0), stop=(k == NAUX - 1))
            muomT = cp.tile([LDIM, BC], F32R)
            nc.vector.tensor_copy(muomT, pmw[:, :])

            # ---------------- phase C: encoder ----------------
            y0sb = cp.tile([LDIM, BC], F32R)
            XB = 2
            for jb in range(NBLK // XB):
                xst = sp.tile([D1, XB, 512], F32R, tag="xst")
                nc.sync.dma_start(
                    out=xst, in_=xsT_d[:, jb * XB:(jb + 1) * XB, :])
                for jj in range(XB):
                    j = jb * XB + jj
                    rhs = xst[:, jj, :]
                    p1 = pp.tile([128, 2, 512], F32, tag="p1")
                    e1t = sp.tile([128, 2, 512], F32, tag="e1")
                    h1 = sp.tile([128, 2, 512], F32R, tag="h1")
                    for mo in range(2):
                        nc.tensor.matmul(p1[:, mo, :],
                                         w1es[:, mo * 128:(mo + 1) * 128],
                                         rhs, start=True, stop=True)
                    nc.scalar.copy(e1t[:], p1[:, :, :])
                    nc.vector.scalar_tensor_tensor(h1[:], e1t[:], 0.01,
                                                   e1t[:], OP.mult, OP.max)
                    p2 = pp.tile([128, 2, 512], F32, tag="p2")
                    e2t = sp.tile([128, 2, 512], F32, tag="e2")
                    h2 = sp.tile([128, 2, 512], F32R, tag="h2")
                    for mo in range(2):
                        for ki in range(2):
                            nc.tensor.matmul(p2[:, mo, :],
                                             w2_sb[:, ki * 2 + mo, :],
                                             h1[:, ki, :],
                                             start=(ki == 0), stop=False)
                        nc.tensor.matmul(p2[:, mo, :],
                                         b2r_sb[:, mo * 128:(mo + 1) * 128],
                                         ones_row[:], start=False, stop=True)
                    nc.scalar.copy(e2t[:], p2[:, :, :])
                    nc.vector.scalar_tensor_tensor(h2[:], e2t[:], 0.01,
                                                   e2t[:], OP.mult, OP.max)
                    p3 = pp.tile([LDIM, 512], F32, tag="p3")
                    for ki in range(2):
                        nc.tensor.matmul(p3[:, :], w3_sb[:, ki, :],
                                         h2[:, ki, :],
                                         start=(ki == 0), stop=(ki == 1))
                    yt = sp.tile([LDIM, 512], F32, tag="yenc")
                    nc.scalar.copy(yt[:], p3[:, :])
                    nc.sync.dma_start(out=yencT_o[:, j, :], in_=yt[:])
                    if j == 0:
                        nc.vector.tensor_copy(y0sb, yt[:])

            # ---------------- phase D: closed-form Koopman rotation ----------
            muom_b = []
            y0_b = []
            identR64 = identR_t[0:LDIM, 0:LDIM]
            for c in range(RCH):
                pt = pp.tile([128, 128], F32, tag="pa1")
                nc.tensor.transpose(pt[:, 0:LDIM].bitcast(F32R),
                                    muomT[:, c * 128:(c + 1) * 128], identR64)
                mb = cp.tile([128, LDIM], F32, tag=f"muomb{c}")
                nc.vector.tensor_copy(mb, pt[:, 0:LDIM])
                muom_b.append(mb)
                pt2 = pp.tile([128, 128], F32, tag="pa1")
                nc.tensor.transpose(pt2[:, 0:LDIM].bitcast(F32R),
                                    y0sb[:, c * 128:(c + 1) * 128], identR64)
                yb = cp.tile([128, LDIM], F32, tag=f"y0b{c}")
                nc.vector.tensor_copy(yb, pt2[:, 0:LDIM])
                y0_b.append(yb)

            for c in range(RCH):
                y00 = y0_b[c][:, 0::2]           # [128, 32]
                y01 = y0_b[c][:, 1::2]
                mu = muom_b[c][:, 0::2]
                om = muom_b[c][:, 1::2]

                stage = sp.tile([128, T, LDIM], F32R, tag="stage", bufs=1)
                for th in range(2):
                    tvb = tv_sb[:, th * 32:(th + 1) * 32].unsqueeze(2) \
                        .broadcast_to([128, 32, 32])
                    omb = om.unsqueeze(1).broadcast_to([128, 32, 32])
                    mub = mu.unsqueeze(1).broadcast_to([128, 32, 32])
                    y00b = y00.unsqueeze(1).broadcast_to([128, 32, 32])
                    y01b = y01.unsqueeze(1).broadcast_to([128, 32, 32])
                    ang = rp.tile([128, 32, 32], F32, tag="ang")
                    nc.vector.tensor_mul(ang, omb, tvb)
                    ang2 = rp.tile([128, 32, 32], F32, tag="ang2")
                    nc.vector.tensor_mul(ang2, mub, tvb)
                    aw = rp.tile([128, 32, 32], F32, tag="aw")
                    nc.scalar.activation(aw, ang, AF.Abs)
                    ss = rp.tile([128, 32, 32], F32, tag="ss")
                    nc.scalar.activation(ss, ang, AF.Sin)
                    cc = rp.tile([128, 32, 32], F32, tag="cc")
                    nc.scalar.activation(cc, aw, AF.Sin, bias=hpib[:, :],
                                         scale=-1.0)
                    ee = rp.tile([128, 32, 32], F32, tag="ee")
                    nc.scalar.activation(ee, ang2, AF.Exp)
                    ec = rp.tile([128, 32, 32], F32, tag="ang")
                    nc.vector.tensor_mul(ec, ee, cc)
                    es = rp.tile([128, 32, 32], F32, tag="ang2")
                    nc.vector.tensor_mul(es, ee, ss)
                    m1 = rp.tile([128, 32, 32], F32, tag="m1")
                    nc.vector.tensor_mul(m1, ec, y00b)
                    m2 = rp.tile([128, 32, 32], F32, tag="m2")
                    nc.vector.tensor_mul(m2, es, y01b)
                    m3 = rp.tile([128, 32, 32], F32, tag="m1")
                    nc.vector.tensor_mul(m3, es, y00b)
                    m4 = rp.tile([128, 32, 32], F32, tag="m2")
                    nc.vector.tensor_mul(m4, ec, y01b)
                    tsl = slice(th * 32, (th + 1) * 32)
                    nc.vector.tensor_sub(stage[:, tsl, 0::2], m1, m2)
                    nc.vector.tensor_add(stage[:, tsl, 1::2], m3, m4)
                nc.sync.dma_start(
                    out=yl_o[c * 128:(c + 1) * 128, :],
                    in_=stage[:].rearrange("p t d -> p (t d)"))

                # x_pred: transpose yl tiles to d-major, then Cw matmul
                stg = stage[:].rearrange("p t d -> p (t d)")
                xpt = sp.tile([32, 8, 4, 128], F32, tag="xpt", bufs=1)
                for qd4 in range(8):
                    ptq = pp.tile([128, 512], F32, tag="pa1")
                    for i in range(4):
                        tau = qd4 * 4 + i
                        nc.tensor.transpose(
                            ptq[:, i * 128:(i + 1) * 128].bitcast(F32R),
                            stg[:, tau * 128:(tau + 1) * 128], identR)
                    ylt = sp.tile([128, 512], F32R, tag="ylt", bufs=1)
                    nc.vector.tensor_copy(ylt[:], ptq[:, :])
                    pxp = pp.tile([32, 512], F32, tag="pa2")
                    nc.tensor.matmul(pxp[:, :], cw2_sb[:], ylt[:],
                                     start=True, stop=True)
                    nc.scalar.copy(
                        xpt[:, qd4, :, :],
                        pxp[:, :].rearrange("p (t b) -> p t b", t=4))
                nc.sync.dma_start(
                    out=xpT_o[:, :, c * 128:(c + 1) * 128],
                    in_=xpt[:].rearrange("p q t b -> p (q t) b"))
    nc.finalize()
    return nc


def _host_prep(inputs):
    f32 = np.float32
    xs = np.asarray(inputs["xs"], f32)
    w1g = np.asarray(inputs["enc_W1"], f32) * np.asarray(inputs["enc_bn_gamma"], f32)
    b1h = (np.asarray(inputs["enc_b1"], f32)
           + np.asarray(inputs["enc_W1"], f32) @ np.asarray(inputs["enc_bn_beta"], f32))
    w1e = np.ascontiguousarray(w1g.T)                       # [16, 256]
    w2 = np.asarray(inputs["enc_W2"], f32)
    w2sb = np.empty((128, 4, 128), f32)
    for ki in range(2):
        for mo in range(2):
            w2sb[:, ki * 2 + mo, :] = w2[mo * 128:(mo + 1) * 128,
                                         ki * 128:(ki + 1) * 128].T
    w3 = np.asarray(inputs["enc_W3"], f32) * np.asarray(inputs["enc_scale"], f32)[:, None]
    w3sb = np.empty((128, 2, LDIM), f32)
    for ki in range(2):
        w3sb[:, ki, :] = w3[:, ki * 128:(ki + 1) * 128].T
    w1a = np.asarray(inputs["aux_W1"], f32) * np.asarray(inputs["aux_bn_gamma"], f32)[:, None, :]
    w1asb = np.ascontiguousarray(w1a.reshape(NAUX * AH, DIM).T)   # [16, 4096]
    b1a = (np.asarray(inputs["aux_b1"], f32)
           + np.einsum("kji,ki->kj", np.asarray(inputs["aux_W1"], f32),
                       np.asarray(inputs["aux_bn_beta"], f32)))
    w2asb = np.ascontiguousarray(
        np.asarray(inputs["aux_W2"], f32).transpose(2, 0, 1))     # [128, 32, 128]
    w3adt = (np.asarray(inputs["aux_W3"], f32)
             * np.asarray(inputs["aux_scale"], f32)[:, :, None] * DT)
    w3asb = np.zeros((AH, NAUX, LDIM), f32)
    for k in range(NAUX):
        w3asb[:, k, 2 * k] = w3adt[k, 0, :]
        w3asb[:, k, 2 * k + 1] = w3adt[k, 1, :]
    cw = np.asarray(inputs["Cw"], f32)                      # [16, 64]
    cw2 = np.zeros((128, 32), f32)
    cw2[0:64, 0:16] = cw.T
    cw2[64:128, 16:32] = cw.T
    tv = np.broadcast_to(np.arange(T, dtype=f32), (128, T)).copy()

    shared = dict(
        w1e=w1e, b1r=np.ascontiguousarray(b1h.reshape(1, H)), w2=w2sb,
        b2r=np.ascontiguousarray(np.asarray(inputs["enc_b2"], f32).reshape(1, H)),
        w3=w3sb, w1a=w1asb,
        b1ar=np.ascontiguousarray(b1a.reshape(1, NAUX * AH)),
        w2a=w2asb,
        b2ar=np.ascontiguousarray(np.asarray(inputs["aux_b2"], f32).reshape(1, NAUX * AH)),
        w3a=w3asb, cw2=cw2, tv=tv, idm=np.eye(128, dtype=f32),
        onesr=np.ones((1, 512), f32))
    in_maps = []
    for c in range(NCORES):
        xc = xs[c * BC:(c + 1) * BC]                        # [512, 64, 16]
        xsT = np.empty((D1, T, BC), f32)
        xsT[0:DIM] = xc.transpose(2, 1, 0)
        xsT[DIM] = 1.0
        xsN1 = np.concatenate(
            [xc.reshape(COLS, DIM), np.ones((COLS, 1), f32),
             np.zeros((COLS, 1), f32)], axis=1)
        m = dict(shared)
        m["xsT"] = xsT
        m["xsN1"] = np.ascontiguousarray(xsN1)
        in_maps.append(m)
    return in_maps


def _assemble(inputs, results):
    f32 = np.float32
    xs = np.asarray(inputs["xs"], f32)
    y = np.empty((B, T, DIM + LDIM), f32)
    y_pred = np.empty((B, T, DIM + LDIM), f32)
    y[:, :, :DIM] = xs
    for c in range(NCORES):
        r = results[c]
        sl = slice(c * BC, (c + 1) * BC)
        y[sl, :, DIM:] = r["yencT"].reshape(LDIM, T, BC).transpose(2, 1, 0)
        y_pred[sl, :, DIM:] = r["yl"].reshape(BC, T, LDIM)
        xp = r["xpT"].reshape(2, 16, 32, BC).transpose(3, 2, 0, 1) \
            .reshape(BC, T, DIM)
        y_pred[sl, :, :DIM] = xp
    y_pred[:, 0, :DIM] = xs[:, 0, :]
    return y, y_pred


_NC_CACHE = {}


def kernel(**inputs):
    if "nc" not in _NC_CACHE:
        _NC_CACHE["nc"] = build()
    nc = _NC_CACHE["nc"]
    in_maps = _host_prep(inputs)
    res = bass_utils.run_bass_kernel_spmd(nc, in_maps,
                                          core_ids=list(range(NCORES)))
    return _assemble(inputs, res.results)



# revision 43
# speedup vs baseline: 1.7176x; 1.7176x over previous
"""Trainium2 Bass kernel for nn_DENIS_JBF (Koopman Jordan-block forecast).

v2 design (pure data parallel, 8 cores, Bc = 512 batch per core):
  - BatchNorm batch stats via per-shard Gram sums on PE (bf16 inputs),
    one tiny AllReduce, BN folded into L1 weights / x0 on device.
  - MLP evictions fused: leaky-relu + bias + PSUM->SBUF in ONE op —
    L1 layers via DVE scalar_tensor_tensor (bias rides the matmul
    ones-row), L2 layers via ACT Prelu (per-partition bias operand,
    parametric_relu lives in every act table so no table loads).
  - Encoder L3 outputs of two consecutive t-blocks stacked into one
    [128, 512] PSUM tile -> single eviction, bf16 DMA out.
  - Koopman closed form: rotation factors sin/cos/exp on ACT with
    table-grouped ordering, products on DVE in packed bf16 (2x mode)
    with a [t, comp, pair] stage layout (host un-permutes), the
    angle outer-products on the otherwise-idle Pool engine.
  - x_pred: PE transposes of bf16 stage cols + block-diag Cw matmul,
    4-way stacked PSUM evictions, bf16 out.
  - All bulk DMA issued from the gpsimd queue (cheap dispatch).
"""

import os
import sys

import numpy as np

for _p in ("/opt/trn_rl_repo", "/root/.axon_site/_ro/trn_rl_repo"):
    if os.path.isdir(_p) and _p not in sys.path:
        sys.path.insert(0, _p)

import concourse.bass as bass
from concourse import bacc
import concourse.mybir as mybir
import concourse.tile as tile
from concourse import bass_utils

try:
    from ml_dtypes import bfloat16 as np_bf16
except Exception:  # pragma: no cover
    import jax.numpy as _jnp
    np_bf16 = _jnp.bfloat16

F32 = mybir.dt.float32
F32R = mybir.dt.float32r
BF16 = mybir.dt.bfloat16
AF = mybir.ActivationFunctionType
OP = mybir.AluOpType
AX = mybir.AxisListType

NCORES = 8
B, T, DIM, LDIM, NAUX = 4096, 64, 16, 64, 32
H, AH = 256, 128
DT = 0.01
EPS = 1e-5
BC = B // NCORES            # 512
COLS = BC * T               # 32768 rows for stats
NBLK = T                    # 64 encoder col-blocks (one per t)
RCH = BC // 128             # 4 rotation b-chunks
PI = float(np.pi)
D1 = DIM + 1                # 17: input dim + ones row (bias-in-matmul)
ALPHA = 0.01                # leaky-relu slope


def build():
    nc = bacc.Bacc(None)

    # ---------------- DRAM I/O ----------------
    xsT_d = nc.dram_tensor("xsT", [D1, T, BC], F32R, kind="ExternalInput")
    xsg_d = nc.dram_tensor("xsg", [4, 128, 64 * 18], BF16, kind="ExternalInput")
    x0b_d = nc.dram_tensor("x0b", [128, RCH, 18], BF16, kind="ExternalInput")
    w1e_d = nc.dram_tensor("w1e", [DIM, H], F32, kind="ExternalInput")
    b1r_d = nc.dram_tensor("b1r", [1, H], F32, kind="ExternalInput")
    w2_d = nc.dram_tensor("w2", [128, 4, 128], F32R, kind="ExternalInput")
    b2c_d = nc.dram_tensor("b2c", [128, 2], F32, kind="ExternalInput")
    w3_d = nc.dram_tensor("w3", [128, 4, 128], F32R, kind="ExternalInput")
    w1a_d = nc.dram_tensor("w1a", [D1, NAUX * AH], F32R, kind="ExternalInput")
    w2a_d = nc.dram_tensor("w2a", [AH, NAUX, AH], F32R, kind="ExternalInput")
    b2ac_d = nc.dram_tensor("b2ac", [128, NAUX], F32, kind="ExternalInput")
    w3a_d = nc.dram_tensor("w3a", [128, NAUX, LDIM], F32R, kind="ExternalInput")
    cw2_d = nc.dram_tensor("cw2", [128, 32], BF16, kind="ExternalInput")
    tv_d = nc.dram_tensor("tv", [128, T], F32, kind="ExternalInput")
    id_d = nc.dram_tensor("idm", [128, 128], F32R, kind="ExternalInput")

    yenc_o = nc.dram_tensor("yenc", [128, NBLK // 2, BC], BF16,
                            kind="ExternalOutput")
    yl_o = nc.dram_tensor("yl", [RCH, 128, T * LDIM], BF16,
                          kind="ExternalOutput")
    xp_o = nc.dram_tensor("xp", [RCH, 4, 64, 512], BF16,
                          kind="ExternalOutput")

    stat_in = nc.dram_tensor("stat_in", [2, 18, 18], F32)
    stat_out = nc.dram_tensor("stat_out", [NCORES, 2, 18, 18], F32,
                              addr_space="Shared")

    with tile.TileContext(nc) as tc:
        with tc.tile_pool(name="consts", bufs=1) as cp, \
             tc.tile_pool(name="psum", bufs=1, space="PSUM") as pp, \
             tc.tile_pool(name="stream", bufs=2) as sp, \
             tc.tile_pool(name="rot", bufs=1) as rp, \
             tc.tile_pool(name="smalls", bufs=1) as smp:

            # ------------- phase A: batch-stat Gram sums (bf16) -------------
            # All stat loads dispatched first (SP queue is in-order; nothing
            # may sit ahead of them), x0 collective fires early, trajectory
            # collective queues right behind it on the collective cores.
            ch0 = smp.tile([128, RCH, 18], BF16, tag="statch0")
            nc.sync.dma_start(out=ch0, in_=x0b_d[:, :, :])
            chs = []
            for i in range(4):
                ch = sp.tile([128, 64 * 18], BF16, tag="statch", bufs=4,
                             name=f"statch_{i}")
                nc.sync.dma_start(out=ch, in_=xsg_d[i, :, :])
                chs.append(ch)
            pg0 = pp.tile([18, 18], F32, tag="p2", bufs=2)
            for g in range(RCH):
                nc.tensor.matmul(pg0[:, :], ch0[:, g, :], ch0[:, g, :],
                                 start=(g == 0), stop=(g == RCH - 1))
            gB = smp.tile([18, 18], F32, tag="gB")
            nc.vector.tensor_copy(gB, pg0[:, :])
            nc.scalar.dma_start(out=stat_in[0, :, :], in_=gB[:])

            # big traj gram: matmuls as soon as chunks land; ONE combined
            # AllGather for both stat sets (two would serialize on the
            # collective cores at ~15.5us each)
            pg = pp.tile([18, 18], F32, tag="p1", bufs=2)
            NG = 64
            for i in range(4):
                chv = chs[i][:].rearrange("p (g c) -> p g c", g=NG)
                for g in range(NG):
                    idx = i * NG + g
                    nc.tensor.matmul(pg[:, :], chv[:, g, :], chv[:, g, :],
                                     start=(idx == 0), stop=(idx == 255))
            gA = smp.tile([18, 18], F32, tag="gA")
            nc.vector.tensor_copy(gA, pg[:, :])
            nc.scalar.dma_start(out=stat_in[1, :, :], in_=gA[:])
            nc.gpsimd.collective_compute(
                "AllGather", OP.bypass, replica_groups=[list(range(NCORES))],
                ins=[stat_in[:, :, :]], outs=[stat_out[:, :, :, :]])

            # aux-critical constants (land before fold0 completes)
            w1as = cp.tile([D1, NAUX * AH], F32R)
            nc.sync.dma_start(out=w1as, in_=w1a_d[:, :])
            w2a_sb = cp.tile([AH, NAUX, AH], F32R)
            nc.sync.dma_start(out=w2a_sb, in_=w2a_d[:, :, :])
            identR_t = cp.tile([128, 128], F32R)
            nc.sync.dma_start(out=identR_t, in_=id_d[:, :])
            identR = identR_t[:]
            hpib = cp.tile([128, 1], F32)
            nc.vector.memset(hpib, PI / 2.0)
            epsb = cp.tile([128, 1], F32)
            nc.vector.memset(epsb, EPS)
            x0T = cp.tile([D1, BC], F32)
            nc.sync.dma_start(out=x0T, in_=xsT_d[:, 0, :].bitcast(F32))

            sgb = smp.tile([18, NCORES, 2, 18], F32, tag="sgb")
            nc.scalar.dma_start(out=sgb,
                              in_=stat_out[:, :, :, :].transpose([2, 0, 1, 3]))

            b2ac_sb = cp.tile([128, NAUX], F32)
            nc.sync.dma_start(out=b2ac_sb, in_=b2ac_d[:, :])
            w3ac = cp.tile([128, NAUX, LDIM], F32R)
            nc.sync.dma_start(out=w3ac, in_=w3a_d[:, :, :])

            # encoder/rotation constants (needed from ~encoder start)
            w1e_sb = cp.tile([DIM, H], F32)
            nc.sync.dma_start(out=w1e_sb, in_=w1e_d[:, :])
            b1r_sb = cp.tile([1, H], F32)
            nc.sync.dma_start(out=b1r_sb, in_=b1r_d[:, :])
            w2_sb = cp.tile([128, 4, 128], F32R)
            nc.sync.dma_start(out=w2_sb, in_=w2_d[:, :, :])
            b2c_sb = cp.tile([128, 2], F32)
            nc.sync.dma_start(out=b2c_sb, in_=b2c_d[:, :])
            w3_sb = cp.tile([128, 4, 128], F32R)
            nc.sync.dma_start(out=w3_sb, in_=w3_d[:, :, :])
            cw2_sb = cp.tile([128, 32], BF16)
            nc.sync.dma_start(out=cw2_sb, in_=cw2_d[:, :])
            tv_sb = cp.tile([128, T], F32)
            nc.sync.dma_start(out=tv_sb, in_=tv_d[:, :])

            statsb = smp.tile([18, 2, 18], F32, tag="statsb")
            sba = smp.tile([18, 4, 2, 18], F32, tag="sba")
            nc.vector.tensor_add(sba, sgb[:, 0:4, :, :], sgb[:, 4:8, :, :])
            sbb = smp.tile([18, 2, 2, 18], F32, tag="sbb")
            nc.vector.tensor_add(sbb, sba[:, 0:2, :, :], sba[:, 2:4, :, :])
            nc.vector.tensor_add(statsb, sbb[:, 0, :, :], sbb[:, 1, :, :])

            # ------------- phase A2: fold BN -------------
            def fold(set_idx, n):
                g16 = statsb[0:16, set_idx, 0:16]
                scol = statsb[0:16, set_idx, 16:17]
                m = smp.tile([DIM, 1], F32, tag=f"m{set_idx}")
                nc.vector.tensor_scalar(m, scol, 1.0 / n, None, OP.mult)
                gi = smp.tile([DIM, DIM], F32, tag=f"gi{set_idx}")
                nc.vector.tensor_mul(gi, g16,
                                     identR_t[0:16, 0:16].bitcast(F32))
                qd = smp.tile([DIM, 1], F32, tag=f"qd{set_idx}")
                nc.vector.reduce_sum(qd, gi, axis=AX.X)
                m2 = smp.tile([DIM, 1], F32, tag=f"m2{set_idx}")
                nc.vector.tensor_mul(m2, m, m)
                v = smp.tile([DIM, 1], F32, tag=f"v{set_idx}")
                nc.vector.scalar_tensor_tensor(v, qd, 1.0 / n, m2,
                                               OP.mult, OP.subtract)
                sd = smp.tile([DIM, 1], F32, tag=f"sd{set_idx}")
                nc.scalar.activation(sd, v, AF.Sqrt, bias=epsb[0:DIM, :])
                rs = smp.tile([DIM, 1], F32, tag=f"rs{set_idx}")
                nc.vector.reciprocal(rs, sd)
                return m, rs

            m_0, rs_0 = fold(0, float(B))

            # x0c = rs0'*(x0 - m0') with m0'[16]=0, rs0'[16]=1 so the ones
            # row passes through the same op (engines can't start at p=16)
            m17 = smp.tile([D1, 1], F32, tag="m17")
            nc.vector.memset(m17, 0.0)
            nc.vector.tensor_copy(m17[0:DIM, :], m_0[:])
            rs17 = smp.tile([D1, 1], F32, tag="rs17")
            nc.vector.memset(rs17, 1.0)
            nc.vector.tensor_copy(rs17[0:DIM, :], rs_0[:])
            x0c = cp.tile([D1, BC], F32R)
            nc.vector.scalar_tensor_tensor(
                x0c[:, :], x0T[:], m17[:], rs17[:].broadcast_to([D1, BC]),
                OP.subtract, OP.mult)

            # ------------- phase B: aux nets -> (mu*DT, om*DT) -------------
            def fold1_and_w1es():
                m_all, rs_all = fold(1, float(B * T))
                w1es = cp.tile([D1, H], F32R, name="w1es")
                nc.vector.tensor_mul(w1es[0:DIM, :], w1e_sb,
                                     rs_all[:].broadcast_to([DIM, H]))
                negm = smp.tile([DIM, 2], F32R, tag="negm")
                nc.vector.tensor_scalar(negm,
                                        m_all[:].broadcast_to([DIM, 2]),
                                        -1.0, None, OP.mult)
                pbc = pp.tile([2, H], F32, tag="p3", bufs=2)
                nc.tensor.matmul(pbc[:, :], negm[:], w1es[0:DIM, :],
                                 start=True, stop=True)
                brow = smp.tile([1, H], F32R, tag="brow")
                nc.vector.tensor_add(brow, pbc[0:1, :], b1r_sb[:])
                nc.sync.dma_start(out=w1es[DIM:D1, :], in_=brow[:])
                return w1es

            # leaky-relu eviction from PSUM. The ISA allows only ONE psum
            # operand per DVE op and no Pool max, so the choices are a
            # single ACT Prelu or a 2-op DVE pair; alternate by index to
            # balance engine load (True -> DVE).
            LRELU_DVE_FRAC = 0.556

            def lrelu_evict(dst, psrc, idx):
                if (idx * LRELU_DVE_FRAC) % 1.0 + LRELU_DVE_FRAC >= 1.0:
                    s = sp.tile([128, 512], F32, tag="lrs", bufs=2,
                                name=f"lrs_{idx}")
                    nc.vector.tensor_scalar(s, psrc[:, :], ALPHA, None,
                                            OP.mult)
                    nc.vector.tensor_max(dst, psrc[:, :], s[:])
                else:
                    nc.scalar.activation(dst, psrc[:, :], AF.Prelu,
                                         alpha=ALPHA)

            # software-pipelined so PE never waits on an eviction:
            # PE queue order L1(k+2), L2(k), L3(k-1)
            pmw = pp.tile([LDIM, BC], F32, tag="pmw")
            h1s = {}
            h2s = {}

            def aux_l1(k):
                pa1 = pp.tile([128, BC], F32, tag="p1", bufs=2,
                              name=f"pa1_{k}")
                nc.tensor.matmul(pa1[:, :], w1as[:, k * AH:(k + 1) * AH],
                                 x0c[:], start=True, stop=True)
                h1a = sp.tile([128, BC], F32R, tag="h1a", name=f"h1a_{k}")
                lrelu_evict(h1a[:], pa1, 128 + k)
                h1s[k] = h1a

            def aux_l2(k):
                pa2 = pp.tile([128, BC], F32, tag="p2", bufs=2,
                              name=f"pa2_{k}")
                nc.tensor.matmul(pa2[:, :], w2a_sb[:, k, :], h1s.pop(k)[:],
                                 start=True, stop=True)
                h2a = sp.tile([128, BC], F32R, tag="h2a", name=f"h2a_{k}")
                nc.scalar.activation(h2a[:], pa2[:, :], AF.Prelu,
                                     bias=b2ac_sb[:, k:k + 1], alpha=ALPHA)
                h2s[k] = h2a

            def aux_l3(k):
                nc.tensor.matmul(pmw[:, :], w3ac[:, k, :], h2s.pop(k)[:],
                                 start=(k == 0), stop=(k == NAUX - 1))

            aux_l1(0)
            aux_l1(1)
            for k in range(NAUX):
                if k == 10:
                    w1es = fold1_and_w1es()
                if k + 2 < NAUX:
                    aux_l1(k + 2)
                aux_l2(k)
                if k >= 1:
                    aux_l3(k - 1)
            aux_l3(NAUX - 1)
            muomT = cp.tile([LDIM, BC], F32R)
            nc.vector.tensor_copy(muomT, pmw[:, :])

            # ------------- phase C+D interleaved: encoder, rotation, xp ----
            idb = cp.tile([128, 128], BF16)
            nc.vector.tensor_copy(idb, identR_t[:].bitcast(F32))
            y0sb = cp.tile([LDIM, BC], F32R)
            ysh = {}

            def enc_block_pair(jp):
                """Encoder for t-blocks 2*jp and 2*jp+1, stacked L3 psum."""
                if jp % 2 == 0:
                    ysh["xst"] = sp.tile([D1, 4, 512], F32R, tag="xst",
                                         name=f"xst{jp // 2}")
                    nc.sync.dma_start(
                        out=ysh["xst"], in_=xsT_d[:, 2 * jp:2 * jp + 4, :])
                xstt = ysh["xst"]
                p3 = pp.tile([128, 512], F32, tag="p3", bufs=2)
                for par in range(2):
                    rhs = xstt[:, (jp % 2) * 2 + par, :]
                    h1 = sp.tile([128, 2, 512], F32R, tag="h1")
                    for mo in range(2):
                        p1 = pp.tile([128, 512], F32, tag="p1", bufs=2)
                        nc.tensor.matmul(p1[:, :],
                                         w1es[:, mo * 128:(mo + 1) * 128],
                                         rhs, start=True, stop=True)
                        lrelu_evict(h1[:, mo, :], p1, jp * 4 + par * 2 + mo)
                    h2 = sp.tile([128, 2, 512], F32R, tag="h2")
                    for mo in range(2):
                        p2 = pp.tile([128, 512], F32, tag="p2", bufs=2)
                        for ki in range(2):
                            nc.tensor.matmul(p2[:, :],
                                             w2_sb[:, ki * 2 + mo, :],
                                             h1[:, ki, :],
                                             start=(ki == 0), stop=(ki == 1))
                        nc.scalar.activation(h2[:, mo, :], p2[:, :],
                                             AF.Prelu,
                                             bias=b2c_sb[:, mo:mo + 1],
                                             alpha=ALPHA)
                    for ki in range(2):
                        nc.tensor.matmul(p3[:, :],
                                         w3_sb[:, par * 2 + ki, :],
                                         h2[:, ki, :],
                                         start=(par == 0 and ki == 0),
                                         stop=(par == 1 and ki == 1))
                # stacked eviction (bf16) into a 4-pair staging buffer
                slot = jp % 4
                if slot == 0:
                    ysh["t"] = sp.tile([128, 4, BC], BF16, tag="ystage",
                                       name=f"ystage{jp // 4}")
                ystage = ysh["t"]
                nc.vector.tensor_copy(ystage[:, slot, :], p3[:, :])
                if jp == 0:
                    nc.vector.tensor_copy(y0sb, p3[0:64, :])
                if slot == 3:
                    q = jp // 4
                    nc.sync.dma_start(
                        out=yenc_o[:, 4 * q:4 * q + 4, :], in_=ystage[:])

            # rotation chunk state (tags keyed by chunk parity: chunks 0/1
            # finish before 2/3 start, so 2/3 reuse the same buffers)
            stageT = [None] * RCH

            def rot_trig_gen(c):
                """Transposes to b-major + t<32 angles/trig for chunk c.
                t>=32 factors come from angle addition in rot_mul."""
                w = c % 2
                pt = pp.tile([128, 128], F32, tag="pt")
                nc.tensor.transpose(
                    pt[:, 0:LDIM].bitcast(F32R),
                    muomT[:, c * 128:(c + 1) * 128],
                    identR_t[0:LDIM, 0:LDIM])
                nc.tensor.transpose(
                    pt[:, 64:128].bitcast(F32R),
                    y0sb[:, c * 128:(c + 1) * 128],
                    identR_t[0:LDIM, 0:LDIM])
                mb = rp.tile([128, 128], F32, tag=f"mb{w}")
                nc.vector.tensor_copy(mb, pt[:, 0:128])
                st = {"mb": mb}
                stageT[c] = st
                for nm, lo, hi in (("mu", 0, 64), ("om", 1, 64),
                                   ("y00", 64, 128), ("y01", 65, 128)):
                    tpk = rp.tile([128, 32], F32, tag=f"{nm}{w}")
                    nc.vector.tensor_copy(tpk, mb[:, lo:hi:2])
                    st[nm] = tpk
                yield
                # t-shift bases: sin/cos(32*om), exp(32*mu) (exp in rot_exp)
                om32 = rp.tile([128, 32], F32, tag=f"om32{w}")
                nc.gpsimd.tensor_scalar(om32, st["om"][:], 32.0, None,
                                        OP.mult)
                st["om32"] = om32
                mu32 = rp.tile([128, 32], F32, tag=f"mu32{w}")
                nc.gpsimd.tensor_scalar(mu32, st["mu"][:], 32.0, None,
                                        OP.mult)
                st["mu32"] = mu32
                aw32 = rp.tile([128, 32], F32, tag="aw32")
                nc.scalar.activation(aw32, om32[:], AF.Abs)
                snb = rp.tile([128, 32], F32, tag=f"snb{w}")
                nc.scalar.activation(snb, om32[:], AF.Sin)
                st["snb"] = snb
                cnb = rp.tile([128, 32], F32, tag=f"cnb{w}")
                nc.scalar.activation(cnb, aw32[:], AF.Sin, bias=hpib[:, :],
                                     scale=-1.0)
                st["cnb"] = cnb
                yield
                # t<32 trig
                tvb = tv_sb[:, 0:32].unsqueeze(2).broadcast_to([128, 32, 32])
                omb = st["om"][:].unsqueeze(1).broadcast_to([128, 32, 32])
                mub = st["mu"][:].unsqueeze(1).broadcast_to([128, 32, 32])
                ang = rp.tile([128, 32, 32], F32, tag="ang", bufs=1)
                nc.gpsimd.tensor_mul(ang, omb, tvb)
                ang2 = rp.tile([128, 32, 32], F32, tag=f"ang2{w}")
                nc.gpsimd.tensor_mul(ang2, mub, tvb)
                aw = rp.tile([128, 32, 32], F32, tag="s1a_r")
                nc.scalar.activation(aw, ang, AF.Abs)
                yield
                ss = rp.tile([128, 32, 32], F32, tag=f"ss{w}")
                nc.scalar.activation(ss, ang, AF.Sin)
                yield
                cc = rp.tile([128, 32, 32], F32, tag=f"cc{w}")
                nc.scalar.activation(cc, aw, AF.Sin, bias=hpib[:, :],
                                     scale=-1.0)
                st["ss0"] = ss
                st["cc0"] = cc
                st["ang2"] = ang2
                yield

            def rot_exp_gen(c):
                st = stageT[c]
                w = c % 2
                ee = rp.tile([128, 32, 32], F32, tag=f"ee{w}")
                nc.scalar.activation(ee, st["ang2"], AF.Exp)
                st["ee0"] = ee
                yield
                eb = rp.tile([128, 32], F32, tag=f"eb{w}")
                nc.scalar.activation(eb, st["mu32"][:], AF.Exp)
                st["eb"] = eb
                yield

            def rot_mul_gen(c):
                """bf16 products + stage writes + yl DMA for chunk c."""
                st = stageT[c]
                w = c % 2
                y00h = rp.tile([128, 32], BF16, tag=f"y00h{w}")
                nc.vector.tensor_copy(y00h, st["y00"][:])
                y01h = rp.tile([128, 32], BF16, tag=f"y01h{w}")
                nc.vector.tensor_copy(y01h, st["y01"][:])
                y00b = y00h[:].unsqueeze(1).broadcast_to([128, 32, 32])
                y01b = y01h[:].unsqueeze(1).broadcast_to([128, 32, 32])
                stage = rp.tile([128, T, 64], BF16, tag=f"stage{w}")
                st["stage"] = stage
                # t>=32 factors by angle addition (Pool, bf16)
                snb_b = st["snb"][:].unsqueeze(1).broadcast_to([128, 32, 32])
                cnb_b = st["cnb"][:].unsqueeze(1).broadcast_to([128, 32, 32])
                eb_b = st["eb"][:].unsqueeze(1).broadcast_to([128, 32, 32])
                ss0, cc0, ee0 = st["ss0"], st["cc0"], st["ee0"]
                s1a = rp.tile([128, 32, 32], F32, tag="s1a_r")
                nc.gpsimd.tensor_mul(s1a, ss0[:], cnb_b)
                s1b = rp.tile([128, 32, 32], F32, tag="s1b_r")
                nc.gpsimd.tensor_mul(s1b, cc0[:], snb_b)
                ss1 = rp.tile([128, 32, 32], BF16, tag="ss1_r")
                nc.gpsimd.tensor_add(ss1, s1a[:], s1b[:])
                yield
                c1a = rp.tile([128, 32, 32], F32, tag="s1a_r")
                nc.gpsimd.tensor_mul(c1a, cc0[:], cnb_b)
                c1b = rp.tile([128, 32, 32], F32, tag="s1b_r")
                nc.gpsimd.tensor_mul(c1b, ss0[:], snb_b)
                cc1 = rp.tile([128, 32, 32], BF16, tag="cc1_r")
                nc.gpsimd.tensor_sub(cc1, c1a[:], c1b[:])
                ee1 = rp.tile([128, 32, 32], BF16, tag="ee1_r")
                nc.gpsimd.tensor_mul(ee1, ee0[:], eb_b)
                yield
                for th in range(2):
                    ss = ss0 if th == 0 else ss1
                    cc = cc0 if th == 0 else cc1
                    ee = ee0 if th == 0 else ee1
                    ec = rp.tile([128, 32, 32], BF16, tag="ec")
                    nc.gpsimd.tensor_mul(ec, ee[:], cc[:])
                    es = rp.tile([128, 32, 32], BF16, tag="es")
                    nc.gpsimd.tensor_mul(es, ee[:], ss[:])
                    m1 = rp.tile([128, 32, 32], BF16, tag="m1")
                    nc.gpsimd.tensor_mul(m1, ec[:], y00b)
                    m2 = rp.tile([128, 32, 32], BF16, tag="m2")
                    nc.gpsimd.tensor_mul(m2, es[:], y01b)
                    yield
                    m3 = rp.tile([128, 32, 32], BF16, tag="m3")
                    nc.gpsimd.tensor_mul(m3, es[:], y00b)
                    m4 = rp.tile([128, 32, 32], BF16, tag="m4")
                    nc.gpsimd.tensor_mul(m4, ec[:], y01b)
                    tsl = slice(th * 32, (th + 1) * 32)
                    nc.gpsimd.tensor_sub(
                        stage[:, tsl, 0:32],
                        m1[:], m2[:])
                    nc.gpsimd.tensor_add(
                        stage[:, tsl, 32:64],
                        m3[:], m4[:])
                    yield
                nc.gpsimd.dma_start(
                    out=yl_o[c, :, :],
                    in_=stage[:].rearrange("p t d -> p (t d)"))

            def xp_gen(c):
                """x_pred for chunk c from the bf16 stage."""
                st = stageT[c]
                stg = st["stage"][:].rearrange("p t d -> p (t d)")
                for g4 in range(4):
                    pxs = pp.tile([64, 512], F32, tag="pmw")
                    for qh in range(2):
                        q = g4 * 2 + qh
                        ptb = pp.tile([128, 512], BF16, tag="pt")
                        for i in range(4):
                            tau = q * 4 + i
                            nc.tensor.transpose(
                                ptb[:, i * 128:(i + 1) * 128],
                                stg[:, tau * 128:(tau + 1) * 128],
                                idb[:])
                        ylt = rp.tile([128, 512], BF16, tag="ylt")
                        nc.vector.tensor_copy(ylt[:], ptb[:, :])
                        nc.tensor.matmul(pxs[32 * qh:32 * qh + 32, :],
                                         cw2_sb[:], ylt[:],
                                         start=True, stop=True)
                    xps = rp.tile([64, 512], BF16, tag="xps")
                    nc.vector.tensor_copy(xps[:], pxs[:, :])
                    nc.gpsimd.dma_start(out=xp_o[c, g4, :, :], in_=xps[:])
                    yield

            # emission schedule: encoder pairs with rotation work
            # interleaved at fine granularity (avoids ACT bursts that
            # starve the encoder's PSUM evictions)
            def rot_work():
                for c in (0, 1):
                    yield from rot_trig_gen(c)
                for c in (0, 1):
                    yield from rot_exp_gen(c)
                for c in (0, 1):
                    yield from rot_mul_gen(c)
                    yield from xp_gen(c)
                for c in (2, 3):
                    yield from rot_trig_gen(c)
                for c in (2, 3):
                    yield from rot_exp_gen(c)
                for c in (2, 3):
                    yield from rot_mul_gen(c)
                    yield from xp_gen(c)

            work = rot_work()
            PAIR_BUDGET = [0, 0, 0, 0, 1, 1, 1, 1, 2, 2, 2, 2, 2, 2, 2, 3,
                           3, 3, 3, 3, 3, 3, 3, 3, 3, 3, 3, 3, 4, 4, 4, 4]
            for jp in range(32):
                enc_block_pair(jp)
                for _ in range(PAIR_BUDGET[jp]):
                    try:
                        next(work)
                    except StopIteration:
                        break
            for _ in work:
                pass
